# revision 1
# baseline (speedup 1.0000x reference)
"""Trainium2 Bass kernel for the B-spline (KAN-style) layer:

    out = einsum('bin,ion->bo', b_splines(tanh(x)), coeffs) + x @ base_weight

Key identity: with u = 4*tanh(x) + 7 in (3, 11) (uniform knots at integers
4..10 inside the range), each cubic B-spline basis value is b3(u - n), and the
space spanned by {b3(u-n)}_n over u in (3,11) is exactly {C^2 piecewise cubics
with knots 4..10} = span{1, w, w^2, w^3, relu(k-u)^3 (k=4,5,6), relu(u-k)^3
(k=7..10)} with w = u - 7. So the whole layer collapses to ONE matmul over
K = 11*1024 (x residual + 10 nonlinear planes per input feature; the constant
plane folds into a per-output bias applied at PSUM eviction) with
host-preconvolved weights.

Sharding: data-parallel over batch, 8 cores x 512 rows, weights replicated.
Matmul runs in float32r (~2^-12.5 effective operand rounding, full PE rate).
Plane-block order puts the x residual FIRST so the PE starts on raw DMA'd
x tiles with no elementwise work on the critical path.
"""
import numpy as np

import concourse.bass as bass
import concourse.mybir as mybir
import concourse.tile as tile
from concourse import bacc, bass_utils
from concourse.bass_interp import get_hw_module

B, F, O, NCTRL = 4096, 1024, 1024, 11
NCORES = 8
BS = B // NCORES          # 512 batch rows per core
P = 128
FT = F // P               # 8 feature tiles
OT = O // P               # 8 output tiles
NPLANES = 11              # residual + 10 nonlinear
KT = NPLANES * FT         # 88 k-tiles
F32 = mybir.dt.float32
F32R = mybir.dt.float32r
ACTF = mybir.ActivationFunctionType
ALU = mybir.AluOpType

# plane id -> kind: 0: x residual, 1: w=4t, 2: w^2, 3: w^3,
#                   4..6: relu(k-u)^3 k=4,5,6,  7..10: relu(u-k)^3 k=7..10
RHO_KNOTS = (4, 5, 6)
R_KNOTS = (7, 8, 9, 10)

_cached_program = None


def _build_program():
    nc = bacc.Bacc("TRN2", target_bir_lowering=False, debug=False,
                   enable_asserts=False, num_devices=NCORES)
    # const APs for float biases used by scalar.activation(Relu, bias=...)
    for v in (-1.0, -2.0, -3.0):
        ct = nc.alloc_sbuf_tensor(f"const-float32-{v}", [P, 1], F32)
        nc.gpsimd.memset(ct.ap(), v)
        nc.const_aps.aps[(F32, v)] = ct.ap()
    nc.all_engine_barrier()

    xt_d = nc.dram_tensor("xt", [F, BS], F32R, kind="ExternalInput").ap()
    wk_d = nc.dram_tensor("wk", [NPLANES * F, O], F32R, kind="ExternalInput").ap()
    bias_d = nc.dram_tensor("bias", [P, OT], F32, kind="ExternalInput").ap()
    out_d = nc.dram_tensor("out", [O, BS], F32, kind="ExternalOutput").ap()

    with tile.TileContext(nc) as tc:
        with tc.tile_pool(name="const", bufs=1) as const_pool, \
             tc.tile_pool(name="tpool", bufs=1) as t_pool, \
             tc.tile_pool(name="qpool", bufs=3) as q_pool, \
             tc.tile_pool(name="ppool", bufs=4) as p_pool, \
             tc.tile_pool(name="wpool", bufs=8) as w_pool, \
             tc.tile_pool(name="epool", bufs=8) as e_pool, \
             tc.tile_pool(name="psum", bufs=1, space="PSUM") as psum_pool:

            # x tiles (f32r, fed straight to the matmul as the residual block).
            # gpsimd queue, so the sync queue leads with the wk weight tiles.
            xts = []
            for f in range(FT):
                xt = t_pool.tile([P, BS], F32R, tag=f"xt{f}", name=f"xt{f}")
                nc.gpsimd.dma_start(xt[:], xt_d[f * P:(f + 1) * P, :])
                xts.append(xt)

            bias_t = const_pool.tile([P, OT], F32)
            nc.gpsimd.dma_start(bias_t[:], bias_d)

            psums = [psum_pool.tile([P, BS], F32, tag=f"ps{o}", name=f"ps{o}")
                     for o in range(OT)]

            # HAM warmup: keep the PE busy while the first weight tiles DMA in,
            # so the real matmul stream starts at the warm clock. Writes into
            # psums are discarded by kt=0's start=True.
            warm_f = const_pool.tile([P, BS], F32)
            nc.vector.memset(warm_f[:], 0.0)
            warm = const_pool.tile([P, BS], F32R)
            nc.vector.tensor_copy(warm[:], warm_f[:])
            for i in range(8):
                nc.tensor.matmul(psums[i % OT][:], warm[:, 0:P], warm[:],
                                 start=True, stop=True, skip_group_check=True)

            # t = tanh(x) per feature tile (kept resident)
            ts_ = []
            for f in range(FT):
                tt = t_pool.tile([P, BS], F32, tag=f"t{f}", name=f"t{f}")
                nc.scalar.activation(tt[:], xts[f][:].bitcast(F32), ACTF.Tanh)
                ts_.append(tt)

            def make_plane(p, f):
                """Emit ops producing plane (p, f) as an f32r [P, BS] tile."""
                if p == 0:          # residual: raw x tile, no compute
                    return xts[f]
                t = ts_[f]
                pl = p_pool.tile([P, BS], F32R, tag="plane", name=f"pl{p}_{f}")
                if p == 1:          # w = 4t
                    nc.scalar.activation(pl[:], t[:], ACTF.Copy, scale=4.0)
                elif p == 2:        # w^2 = (4t)^2
                    nc.scalar.activation(pl[:], t[:], ACTF.Square, scale=4.0)
                elif p == 3:        # w^3 = (64*t^2)*t
                    t2 = q_pool.tile([P, BS], F32, tag="q2", name=f"t2_{f}")
                    nc.scalar.activation(t2[:], t[:], ACTF.Square)
                    nc.vector.scalar_tensor_tensor(pl[:], t2[:], 64.0, t[:],
                                                   ALU.mult, ALU.mult)
                else:
                    if p <= 6:      # relu(k-u)^3 = relu(-4t + (k-7))^3
                        k = RHO_KNOTS[p - 4]
                        sc, bi = -4.0, float(k - 7)
                    else:           # relu(u-k)^3 = relu(4t + (7-k))^3
                        k = R_KNOTS[p - 7]
                        sc, bi = 4.0, float(7 - k)
                    q = q_pool.tile([P, BS], F32, tag="q", name=f"q{p}_{f}")
                    nc.scalar.activation(q[:], t[:], ACTF.Relu, scale=sc, bias=bi)
                    q2 = q_pool.tile([P, BS], F32, tag="q2", name=f"q2_{p}_{f}")
                    nc.scalar.activation(q2[:], q[:], ACTF.Square)
                    nc.vector.tensor_mul(pl[:], q2[:], q[:])
                return pl

            for kt in range(KT):
                p, f = divmod(kt, FT)
                pl = make_plane(p, f)
                wt = w_pool.tile([P, O], F32R, tag="wk", name=f"wk{kt}")
                nc.sync.dma_start(wt[:], wk_d[kt * P:(kt + 1) * P, :])
                for o in range(OT):
                    nc.tensor.matmul(psums[o][:], wt[:, o * P:(o + 1) * P], pl[:],
                                     start=(kt == 0), stop=(kt == KT - 1))

            # evict: out[o] = psum[o] + bias[:, o], split across Scalar/Vector,
            # out-DMAs split across sync/gpsimd queues
            for o in range(OT):
                ot = e_pool.tile([P, BS], F32, tag=f"evict{o % 2}", name=f"ev{o}")
                if o % 2 == 0:
                    nc.scalar.activation(ot[:], psums[o][:], ACTF.Identity,
                                         bias=bias_t[:, o:o + 1])
                else:
                    nc.vector.tensor_scalar_add(ot[:], psums[o][:],
                                                bias_t[:, o:o + 1])
                eng = (nc.sync, nc.gpsimd, nc.scalar)[o % 3]
                eng.dma_start(out_d[o * P:(o + 1) * P, :], ot[:])

    nc.compile()
    nc.m = get_hw_module(nc.m)
    return nc


def _precompute_weights(coeffs, base_weight):
    """Fold the B-spline basis change into the coefficient tensor.

    b3(v) = (1/6) sum_{j=0..4} C4[j] relu(v-j)^3,  C4 = (1,-4,6,-4,1)
    activation = sum_n coeffs[:,:,n] b3(u-n) = sum_j beta_j relu(u-j)^3
    with u in (3,11):
      j<=3   -> (u-j)^3 exactly        -> monomials in w = u-7 (+ constant)
      4..6   -> (u-j)^3 + relu(j-u)^3  -> monomials + rho_j
      7..10  -> relu(u-j)^3            -> r_j
      j>=11  -> 0
    Returns wk [11*F, O] float32 (plane-block order: residual, w, w^2, w^3,
    rho4..6, r7..10) and bias [P, OT] float32.
    """
    F_, O_, N_ = coeffs.shape
    c = coeffs.astype(np.float64)
    C4 = np.array([1.0, -4.0, 6.0, -4.0, 1.0]) / 6.0
    beta = np.zeros((F_, O_, 15))
    for n in range(N_):
        for j in range(5):
            beta[:, :, n + j] += c[:, :, n] * C4[j]

    const_w = np.zeros((F_, O_))
    mono_w = np.zeros((F_, O_, 3))    # w, w^2, w^3
    rho_w = np.zeros((F_, O_, 3))     # knots 4,5,6 reflected
    r_w = np.zeros((F_, O_, 4))       # knots 7..10
    for j in range(11):
        a = 7.0 - j                   # (u-j)^3 = (w+a)^3
        if j <= 6:
            const_w += beta[:, :, j] * a ** 3
            mono_w[:, :, 0] += beta[:, :, j] * 3 * a ** 2
            mono_w[:, :, 1] += beta[:, :, j] * 3 * a
            mono_w[:, :, 2] += beta[:, :, j]
            if j >= 4:
                rho_w[:, :, j - 4] += beta[:, :, j]
        else:
            r_w[:, :, j - 7] += beta[:, :, j]

    wk = np.concatenate([
        base_weight.astype(np.float64).reshape(F_, O_),
        mono_w.transpose(2, 0, 1).reshape(3 * F_, O_),
        rho_w.transpose(2, 0, 1).reshape(3 * F_, O_),
        r_w.transpose(2, 0, 1).reshape(4 * F_, O_),
    ], axis=0).astype(np.float32)
    bias = const_w.sum(axis=0)                         # [O]
    bias2d = bias.reshape(OT, P).T.astype(np.float32)  # [P, OT], o = j*128 + p
    return np.ascontiguousarray(wk), np.ascontiguousarray(bias2d)


def kernel(x, coeffs, base_weight, grid):
    global _cached_program
    x = np.asarray(x, np.float32)
    coeffs = np.asarray(coeffs, np.float32)
    base_weight = np.asarray(base_weight, np.float32)

    wk, bias2d = _precompute_weights(coeffs, base_weight)
    if _cached_program is None:
        _cached_program = _build_program()
    nc = _cached_program

    in_maps = []
    for c in range(NCORES):
        xs = np.ascontiguousarray(x[c * BS:(c + 1) * BS, :].T)  # [F, BS]
        in_maps.append({"xt": xs, "wk": wk, "bias": bias2d})

    res = bass_utils.run_bass_kernel_spmd(nc, in_maps, core_ids=list(range(NCORES)))
    out = np.empty((B, O), np.float32)
    for c in range(NCORES):
        out[c * BS:(c + 1) * BS, :] = res.results[c]["out"].T
    return out

